# revision 14
# baseline (speedup 1.0000x reference)
"""MoE (8 experts, top-5 Boltzmann gate) Trainium2 kernel.

Strategy: expert-parallel with host-side routing. The gate depends only on
x/Wg/bg, so the host computes softmax + top-5 and gathers, for each expert n,
the tokens that route to it (~2560 of 4096). Core n runs expert n's FFN on
just those tokens (padded to a uniform 128-multiple capacity C), scales rows
by the renormalized gate weight, and the host scatter-adds the per-expert
outputs back to token order. This skips the 3 dropped experts per token
(5/8 of the dense FLOPs) and needs no collectives.

Per core: mm1/mm2 fused over 512-token chunks (ht stays in SBUF), weights
resident in SBUF for the whole kernel, fp16 operands / fp32 PSUM.

DMA layout: every transfer is per-partition contiguous and batched into a
few large triggers (16KB/partition) split across the sync + gpsimd rings;
the scalar queue carries only the ReLU evicts (a DMA trigger stalled on ring
flow control would delay the evict that frees the next PSUM bank).
"""

import numpy as np

# Problem dims (hardcoded per contract)
D_FULL, H_FULL, O_FULL, NEXP = 1024, 4096, 1024, 8
B_FULL = 4096
NCORES = 8
TEMP = float(np.e)
TOPK = 5
N_WARMUP_MM = 24  # dependency-free matmuls bridge input-DMA latency + HAM warm
W1G = 8  # w1 DMA groups (small first groups let mm1 start early)
WG = 4  # w2 DMA groups


def build_moe_bass(C, Cex, D, H, O, num_devices=NCORES):
    """Per-core Bass/Tile program: one expert, C gathered tokens (C % 128 == 0).
    Cex <= C is the real max token count; mm1 skips columns beyond it."""
    from contextlib import ExitStack

    import concourse.bass as bass
    import concourse.tile as tile
    from concourse import bacc, mybir

    f32 = mybir.dt.float32
    f16 = mybir.dt.float16
    P = 128
    assert C % P == 0 and D % P == 0 and H % (2 * P) == 0 and O % 512 == 0
    KD, MH, KH2 = D // P, H // P, H // (2 * P)
    KH = H // P
    NO = O // 512
    T = C // P  # token tiles
    MG, SG = MH // W1G, KH2 // WG  # w1 m-tiles / w2 slabs per DMA group
    assert C - 512 < Cex <= C
    chunks = []  # (tile offset, mm2 width, mm1 width)
    off = 0
    while off < C:
        cs = min(512, C - off)
        chunks.append((off, cs, min(cs, max(1, Cex - off))))
        off += cs
    NCH = len(chunks)

    nc = bacc.Bacc(
        "TRN2", target_bir_lowering=False, debug=False, num_devices=num_devices
    )

    # DRAM I/O (host-packed layouts; all per-partition contiguous)
    xt_d = nc.dram_tensor("xt", [NCH, P, KD, 512], f16, kind="ExternalInput").ap()
    w1_d = nc.dram_tensor("w1t", [P, W1G, MG, KD, P], f16, kind="ExternalInput").ap()
    w2_d = nc.dram_tensor("w2t", [P, WG, SG, 2, O], f16, kind="ExternalInput").ap()
    b1_d = nc.dram_tensor("b1p", [P, MH], f32, kind="ExternalInput").ap()
    b2_d = nc.dram_tensor("b2b", [P, O], f32, kind="ExternalInput").ap()
    wr_d = nc.dram_tensor("wrow", [P, T], f32, kind="ExternalInput").ap()
    out_d = nc.dram_tensor("out", [C, O], f16, kind="ExternalOutput").ap()

    Relu = mybir.ActivationFunctionType.Relu
    Alu = mybir.AluOpType

    with tile.TileContext(nc) as tc, ExitStack() as ctx:
        const = ctx.enter_context(tc.tile_pool(name="const", bufs=1))
        w1p = ctx.enter_context(tc.tile_pool(name="w1", bufs=W1G))
        w2p = ctx.enter_context(tc.tile_pool(name="w2", bufs=WG))
        xtp = ctx.enter_context(tc.tile_pool(name="xt", bufs=2))
        htp = ctx.enter_context(tc.tile_pool(name="ht", bufs=MH + 1))
        b2wp = ctx.enter_context(tc.tile_pool(name="b2w", bufs=2))
        outp = ctx.enter_context(tc.tile_pool(name="outs", bufs=3))
        ps_w = ctx.enter_context(tc.tile_pool(name="ps_w", bufs=1, space="PSUM"))
        ps_1 = ctx.enter_context(tc.tile_pool(name="ps_1", bufs=3, space="PSUM"))
        ps_2 = ctx.enter_context(tc.tile_pool(name="ps_2", bufs=4, space="PSUM"))

        # ---- PE warmup: bridges the input-DMA latency and lifts the HAM
        # clock gate before real work arrives.
        wu = const.tile([P, 512], f16, tag="warmup")
        nc.vector.memset(wu[:], 0.0)
        for i in range(N_WARMUP_MM):
            pw = ps_w.tile([P, 512], f32, tag="ps_wu", name=f"ps_wu{i}")
            nc.tensor.matmul(pw[:], wu[:, 0:P], wu[:], start=True, stop=True)

        # ---- input loads ----
        # sync ring: w1 groups (mm1 critical path), then xt prefetches +
        # output stores; gpsimd ring: xt chunk 0 + biases/gate + w2 groups.
        xt = [None, None]
        xt[0] = xtp.tile([P, KD, 512], f16, tag="xt", name="xt_c0")
        nc.gpsimd.dma_start(xt[0][:], xt_d[0])
        w1 = []
        for g in range(W1G):
            w1g = w1p.tile([P, MG, KD, P], f16, tag="w1", name=f"w1g_{g}")
            nc.sync.dma_start(w1g[:], w1_d[:, g])
            w1.append(w1g)
        b1_sb = const.tile([P, MH], f32)
        nc.gpsimd.dma_start(b1_sb[:], b1_d[:])
        wr_sb = const.tile([P, T], f32)
        nc.gpsimd.dma_start(wr_sb[:], wr_d[:])
        b2_sb = const.tile([P, O], f32)
        nc.gpsimd.dma_start(b2_sb[:], b2_d[:])
        w2 = []
        for g in range(WG):
            w2g = w2p.tile([P, SG, 2, O], f16, tag="w2", name=f"w2g_{g}")
            nc.gpsimd.dma_start(w2g[:], w2_d[:, g])
            w2.append(w2g)

        # ---- chunk loop: mm1 (all H tiles) then mm2 (per token tile) ----
        for ci, (lo, cs, cs1) in enumerate(chunks):
            # prefetch next chunk's tokens
            if ci + 1 < NCH:
                xt[(ci + 1) % 2] = xtp.tile(
                    [P, KD, 512], f16, tag="xt", name=f"xt_c{ci + 1}"
                )
                nc.sync.dma_start(xt[(ci + 1) % 2][:], xt_d[ci + 1])
            xc = xt[ci % 2]

            # mm1: ht[m] = relu(W1_m.T @ x + b1_m), fp16. Columns beyond cs1
            # (zero-gate padding) keep stale-but-finite values; their mm2
            # contribution is zeroed by wrow at evict.
            ht = []
            for m in range(MH):
                ps1 = ps_1.tile([P, 512], f32, tag="ps1", name=f"ps1_{ci}_{m}")
                for k in range(KD):
                    nc.tensor.matmul(
                        ps1[:, 0:cs1],
                        w1[m // MG][:, m % MG, k, :],
                        xc[:, k, 0:cs1],
                        start=(k == 0),
                        stop=(k == KD - 1),
                    )
                h = htp.tile([P, 512], f16, tag="ht", name=f"ht_{ci}_{m}")
                nc.scalar.activation(
                    h[:, 0:cs1], ps1[:, 0:cs1], Relu, bias=b1_sb[:, m : m + 1]
                )
                ht.append(h)

            # mm2 per 128-token tile: out = wrow * (ht.T @ W2 + b2)
            for tl in range(cs // P):
                t = lo // P + tl
                b2w = b2wp.tile([P, O], f32, tag="b2w", name=f"b2w_{t}")
                nc.vector.tensor_scalar_mul(b2w[:], b2_sb[:], wr_sb[:, t : t + 1])
                o_sb = outp.tile([P, O], f16, tag="outs", name=f"o_{t}")
                for oc in range(NO):
                    ps2 = ps_2.tile([P, 512], f32, tag="ps2", name=f"ps2_{t}_{oc}")
                    for kh in range(KH):
                        nc.tensor.matmul(
                            ps2[:],
                            ht[kh][:, tl * P : tl * P + P],
                            w2[kh // (2 * SG)][
                                :, (kh // 2) % SG, kh % 2, oc * 512 : (oc + 1) * 512
                            ],
                            start=(kh == 0),
                            stop=(kh == KH - 1),
                        )
                    nc.vector.scalar_tensor_tensor(
                        o_sb[:, oc * 512 : (oc + 1) * 512],
                        ps2[:],
                        wr_sb[:, t : t + 1],
                        b2w[:, oc * 512 : (oc + 1) * 512],
                        op0=Alu.mult,
                        op1=Alu.add,
                    )
                    nc.sync.dma_start(
                        out_d[t * P : (t + 1) * P, oc * 512 : (oc + 1) * 512],
                        o_sb[:, oc * 512 : (oc + 1) * 512],
                    )

    nc.compile()
    return nc


def _route(x, Wg, bg):
    """Host gate: renormalized top-5 weights + per-expert token lists."""
    logits = x.astype(np.float64) @ Wg.astype(np.float64).T + bg.astype(np.float64)
    p = np.exp(logits / TEMP - logits.max(axis=1, keepdims=True) / TEMP)
    p /= p.sum(axis=1, keepdims=True)
    kth = np.partition(p, NEXP - TOPK, axis=1)[:, NEXP - TOPK : NEXP - TOPK + 1]
    m = (p >= kth).astype(np.float64)
    w = p * m
    w /= w.sum(axis=1, keepdims=True) + 1e-8
    w = w.astype(np.float32)
    idx = [np.nonzero(m[:, n])[0] for n in range(NEXP)]
    return w, idx


def _capacity(idx):
    cmax = max(len(i) for i in idx)
    return max(128, ((cmax + 127) // 128) * 128), max(1, cmax)


def pack_inputs(x, W1, b1, W2, b2, Wg, bg, Bc=None, ncores=NCORES):
    """Host-side routing + shard + relayout. Returns per-core input maps."""
    P = 128
    N, H, D = W1.shape
    O = W2.shape[1]
    KD, MH, KH2 = D // P, H // P, H // (2 * P)
    MG, SG = MH // W1G, KH2 // WG

    x = np.ascontiguousarray(x, np.float32)
    w, idx = _route(x, np.asarray(Wg, np.float32), np.asarray(bg, np.float32))
    C, Cex = _capacity(idx)
    _LAST_C[0] = (C, Cex)
    T = C // P
    NCH = (C + 511) // 512
    CP = NCH * 512

    in_maps = []
    for n in range(ncores):
        ix = idx[n]
        cnt = len(ix)
        xg = np.zeros((CP, D), np.float16)
        xg[:cnt] = x[ix]
        # [ch, p, k, j]: per-chunk, per-partition contiguous
        xt = np.ascontiguousarray(xg.reshape(NCH, 512, KD, P).transpose(0, 3, 2, 1))
        # [p, g, mg, k, q]
        w1t = np.ascontiguousarray(
            W1[n].reshape(W1G, MG, P, KD, P).transpose(4, 0, 1, 3, 2), np.float16
        )
        # [p, g, sg, c, o]
        w2t = np.ascontiguousarray(
            W2[n].T.reshape(WG, SG, 2, P, O).transpose(3, 0, 1, 2, 4), np.float16
        )
        b1p = np.ascontiguousarray(b1[n].reshape(MH, P).T, np.float32)
        b2b = np.ascontiguousarray(np.tile(b2[n][None, :], (P, 1)), np.float32)
        wflat = np.zeros(C, np.float32)
        wflat[:cnt] = w[ix, n]
        wrow = np.ascontiguousarray(wflat.reshape(T, P).T)
        in_maps.append(
            {"xt": xt, "w1t": w1t, "w2t": w2t, "b1p": b1p, "b2b": b2b, "wrow": wrow}
        )
    return in_maps, idx, C


_NC_CACHE = {}
_LAST_C = [None]


def _get_nc(C=None):
    if _LAST_C[0] is not None and (C is None or _LAST_C[0][0] == C):
        C, Cex = _LAST_C[0]
    elif C is None:
        C, Cex = 2688, 2688
    else:
        Cex = C
    key = (C, Cex, D_FULL, H_FULL, O_FULL)
    if key not in _NC_CACHE:
        _NC_CACHE[key] = build_moe_bass(C, Cex, D_FULL, H_FULL, O_FULL)
    return _NC_CACHE[key]


def kernel(x, W1, b1, W2, b2, Wg, bg):
    from concourse.bass_utils import run_bass_kernel_spmd

    x = np.asarray(x)
    in_maps, idx, C = pack_inputs(
        x,
        np.asarray(W1),
        np.asarray(b1),
        np.asarray(W2),
        np.asarray(b2),
        np.asarray(Wg),
        np.asarray(bg),
    )
    nc = _get_nc(C)
    try:
        res = run_bass_kernel_spmd(nc, in_maps, core_ids=list(range(NCORES)))
    except Exception:
        # transient NRT exec-unit failures have been observed to clear on retry
        res = run_bass_kernel_spmd(nc, in_maps, core_ids=list(range(NCORES)))
    out = np.zeros((x.shape[0], O_FULL), np.float32)
    for n in range(NCORES):
        ix = idx[n]
        out[ix] += res.results[n]["out"][: len(ix)].astype(np.float32)
    return out
